# revision 26
# baseline (speedup 1.0000x reference)
"""Trainium2 Bass kernel for nn_ActorCritic loss_fn.

Strategy (batch-major, DVE-scan)
--------------------------------
Both losses are polynomials in 10 masked global sums over the discounted
returns R, values V, log-probs L, entropies E and mask m:

    N    = sum(m)        S1   = sum(m*R)      S2  = sum(m*R^2)
    SV   = sum(m*V)      SRV  = sum(m*R*V)    SV2 = sum(m*V^2)
    SLP  = sum(m*L)      SLPR = sum(m*L*R)    SLPV= sum(m*L*V)
    SE   = sum(m*E)

Layout: batch on SBUF partitions, time along the free dim, TIME-REVERSED
on the host (all sums are order-free, so nothing is un-reversed).  Each
core gets 512 batch columns = 4 partition-blocks of 128; each block's
8192 time steps split into 4 windows of 2048 -> 16 units of (128, 2048)
per core, streamed through a 3-deep input ring.

Engines per unit:
  DVE : the discounted-return scan as ONE native tensor_tensor_scan
        (state = gamma*state + r, fp32 internal state, gamma held as an
        f32 tile so the recurrence coefficient is exact), chained across
        windows via initial=prev_out[:, -1:]; then SEVEN bf16 products
        (mR, mV, mL, mE, mRV, mLR, mLV) in the DVE 2x_1p perf mode
        (2-byte packed operands, ~0.52 ns/col).
  PE  : 7 stat reductions (N,S1,SV,SLP,SRV,SLPR,SLPV) as onehot-column
        matmuls accumulating into one PSUM bank across all 16 units.
  ACT : Square+accum_out for S2/SV2, Copy+accum_out for SE (one column
        per unit; host sums columns).
  GPS : deliberately IDLE - any GpSimd op grabs the SBUF port pair that
        the DVE 2x perf mode needs (exclusive per-instruction lock) and
        stalls the products far more than it contributes (measured).

Rewards travel as fp8e4 (they only feed the scan, whose state is fp32);
everything else bf16 so the DVE products keep 2x mode.  Per-product
semaphores let PE/ACT trail the DVE by one product rather than one unit
(tight pipeline tail), and unit 0's five tensors get dedicated
semaphores so the first scan starts as soon as rewards land.

Raw Bass with manual semaphores (this walrus build allows one sync wait
per instruction -> standalone wait_ge).  Final scalar math on host in
float64 from the 10 sums.
"""

import numpy as np
from contextlib import ExitStack

GAMMA = 0.99
ALPHA = 0.01
EPS = 1e-8

T = 8192
B = 4096
NCORES = 8
BL = B // NCORES          # 512 batch columns per core
P = 128                   # partition dim (batch block)
NBLK = BL // P            # 4 batch blocks
W = 2048                  # time window (free dim per unit)
NWIN = T // W             # 4 windows per block
NUNIT = NBLK * NWIN       # 16 units, u = j*NWIN + w
NCH = W // 512            # 4 matmul chunks per unit (moving max 512)

# dtypes for rewards / entropies ("bf16" or "fp8")
R_DT = "fp8"
E_DT = "bf16"

PE_STATS = ("N", "S1", "SV", "SLP", "SRV", "SLPR", "SLPV")
NPE = len(PE_STATS)

_cache = {}


def _build_program():
    import concourse.bass as bass
    import concourse.mybir as mybir
    import ml_dtypes

    dt = mybir.dt
    f32 = dt.float32
    bf16 = dt.bfloat16
    fp8 = dt.float8e4
    mult = mybir.AluOpType.mult
    add = mybir.AluOpType.add
    Square = mybir.ActivationFunctionType.Square
    Copy = mybir.ActivationFunctionType.Copy

    r_dt = fp8 if R_DT == "fp8" else bf16
    e_dt = fp8 if E_DT == "fp8" else bf16

    nc = bass.Bass()
    r_d = nc.dram_tensor("rewards", [NUNIT * P, W], r_dt, kind="ExternalInput")
    v_d = nc.dram_tensor("value_estimates", [NUNIT * P, W], bf16, kind="ExternalInput")
    l_d = nc.dram_tensor("log_probs", [NUNIT * P, W], bf16, kind="ExternalInput")
    e_d = nc.dram_tensor("entropies", [NUNIT * P, W], e_dt, kind="ExternalInput")
    m_d = nc.dram_tensor("to_include", [NUNIT * P, W], bf16, kind="ExternalInput")
    pes_d = nc.dram_tensor("pe_stats", [NPE, BL], f32, kind="ExternalOutput")
    cols_d = nc.dram_tensor("acc_cols", [P, 3 * NUNIT], f32, kind="ExternalOutput")

    # onehot matrix for stat matmuls: oneh[:, j*NPE + j] = 1
    oneh_np = np.zeros((P, NPE * NPE), dtype=np.float32)
    for j in range(NPE):
        oneh_np[:, j * NPE + j] = 1.0
    oneh_d = nc.inline_tensor(oneh_np.astype(ml_dtypes.bfloat16), "onehmat")
    # gamma tile for the scan (f32 so the recurrence coefficient is exact)
    gam_d = nc.inline_tensor(np.full((P, W), GAMMA, dtype=np.float32), "gammat")

    with ExitStack() as ctx:
        def sb(name, shape, dtype):
            return ctx.enter_context(nc.sbuf_tensor(name, list(shape), dtype))

        oneh_sb = sb("oneh_sb", (P, NPE * NPE), bf16)
        gam_sb = sb("gam_sb", (P, W), f32)
        r_in = [sb(f"r_in{i}", (P, W), r_dt) for i in range(3)]
        v_in = [sb(f"v_in{i}", (P, W), bf16) for i in range(3)]
        l_in = [sb(f"l_in{i}", (P, W), bf16) for i in range(3)]
        e_in = [sb(f"e_in{i}", (P, W), e_dt) for i in range(3)]
        m_in = [sb(f"m_in{i}", (P, W), bf16) for i in range(3)]
        R_t = [sb(f"R_t{i}", (P, W), bf16) for i in range(2)]
        mR = [sb(f"mR{i}", (P, W), bf16) for i in range(2)]
        mV = [sb(f"mV{i}", (P, W), bf16) for i in range(2)]
        mL = [sb(f"mL{i}", (P, W), bf16) for i in range(2)]
        mRV = [sb(f"mRV{i}", (P, W), bf16) for i in range(2)]
        mLR = [sb(f"mLR{i}", (P, W), bf16) for i in range(2)]
        mLV = [sb(f"mLV{i}", (P, W), bf16) for i in range(2)]
        mE = [sb(f"mE{i}", (P, W), bf16) for i in range(2)]
        sq = sb("sq", (P, W), bf16)
        cols = sb("cols", (P, 3 * NUNIT), f32)
        stats_sb = sb("stats_sb", (NPE, BL), f32)
        st_ps = ctx.enter_context(nc.psum_tensor("st_ps", [NPE, BL], f32))

        with nc.Block() as block, \
                nc.semaphore("const_sem") as const_sem, \
                nc.semaphore("rsem0") as rsem0, \
                nc.semaphore("rsem0b") as rsem0b, \
                nc.semaphore("msem0") as msem0, \
                nc.semaphore("vsem0") as vsem0, \
                nc.semaphore("lsem0") as lsem0, \
                nc.semaphore("esem0") as esem0, \
                nc.semaphore("dr0") as dr0, \
                nc.semaphore("dr1") as dr1, \
                nc.semaphore("dr2") as dr2, \
                nc.semaphore("dve_p8") as dve_p8, \
                nc.semaphore("pe_stat") as pe_stat, \
                nc.semaphore("act_done") as act_done, \
                nc.semaphore("act_se") as act_se, \
                nc.semaphore("act_fin") as act_fin, \
                nc.semaphore("dma_out") as dma_out:
            dring = (dr0, dr1, dr2)
            # per-slot completion thresholds (unit 0's rewards use rsem0)
            thresh = {}
            cnt = [0, 0, 0]
            for u in range(NUNIT):
                cnt[u % 3] += 0 if u == 0 else 80
                thresh[u] = cnt[u % 3]

            @block.sync
            def _(sync):
                # consts + unit-0 rewards first and ALONE, so the first scan
                # is not queued behind the bulk prefetch (queues share fairly)
                sync.dma_start(out=gam_sb[:], in_=gam_d[:]).then_inc(const_sem, 16)
                sync.dma_start(out=oneh_sb[:], in_=oneh_d[:]).then_inc(const_sem, 16)
                sync.dma_start(out=r_in[0][0:64, :], in_=r_d[0:64, :]).then_inc(rsem0, 16)
                sync.dma_start(out=r_in[0][64:128, :], in_=r_d[64:128, :]).then_inc(rsem0b, 16)
                for dst, src, sem in ((m_in[0], m_d, msem0), (v_in[0], v_d, vsem0),
                                      (l_in[0], l_d, lsem0), (e_in[0], e_d, esem0)):
                    sync.dma_start(out=dst[:], in_=src[0:P, :]).then_inc(sem, 16)
                sync.dma_start(out=r_in[1][:], in_=r_d[P:2 * P, :]).then_inc(dr1, 16)
                for dst, src in ((v_in[1], v_d), (l_in[1], l_d),
                                 (e_in[1], e_d), (m_in[1], m_d)):
                    sync.dma_start(out=dst[:], in_=src[P:2 * P, :]).then_inc(dr1, 16)
                for u in range(2, NUNIT):
                    if u >= 3:
                        sync.wait_ge(dve_p8, 8 * (u - 2))
                        sync.wait_ge(pe_stat, u - 2)
                    sl = u % 3
                    rows = slice(u * P, (u + 1) * P)
                    for dst, src in ((r_in[sl], r_d), (v_in[sl], v_d),
                                     (l_in[sl], l_d), (e_in[sl], e_d),
                                     (m_in[sl], m_d)):
                        sync.dma_start(out=dst[:], in_=src[rows, :]) \
                            .then_inc(dring[sl], 16)
                sync.wait_ge(act_fin, 1)
                sync.dma_start(out=pes_d[:], in_=stats_sb[:]).then_inc(dma_out, 16)
                sync.wait_ge(act_done, 2 * NUNIT)
                sync.wait_ge(act_se, NUNIT)
                sync.dma_start(out=cols_d[:], in_=cols[:]).then_inc(dma_out, 16)
                sync.wait_ge(dma_out, 32)

            @block.vector
            def _(dve):
                dve.wait_ge(const_sem, 32)   # both const DMAs (order across queues not guaranteed)
                for u in range(NUNIT):
                    sl = u % 3
                    pr = u % 2
                    if u == 0:
                        dve.wait_ge(rsem0, 16)
                        dve.wait_ge(rsem0b, 16)
                    else:
                        dve.wait_ge(dring[sl], thresh[u])
                    if u >= 2:
                        # product ring WAR: PE stats / ACT reads of u-2 done
                        dve.wait_ge(pe_stat, u - 1)
                        dve.wait_ge(act_done, 2 * (u - 1))
                        dve.wait_ge(act_se, u - 1)
                    init = 0.0 if u % NWIN == 0 else R_t[(u - 1) % 2][:, W - 1:W]
                    dve.tensor_tensor_scan(out=R_t[pr][:], data0=gam_sb[:],
                                           data1=r_in[sl][:], initial=init,
                                           op0=mult, op1=add).then_inc(dve_p8, 1)
                    if u == 0:
                        dve.wait_ge(msem0, 16)
                    dve.tensor_tensor(out=mR[pr][:], in0=m_in[sl][:], in1=R_t[pr][:], op=mult).then_inc(dve_p8, 1)
                    if u == 0:
                        dve.wait_ge(vsem0, 16)
                    dve.tensor_tensor(out=mV[pr][:], in0=m_in[sl][:], in1=v_in[sl][:], op=mult).then_inc(dve_p8, 1)
                    if u == 0:
                        dve.wait_ge(lsem0, 16)
                    dve.tensor_tensor(out=mL[pr][:], in0=m_in[sl][:], in1=l_in[sl][:], op=mult).then_inc(dve_p8, 1)
                    if u == 0:
                        dve.wait_ge(esem0, 16)
                    dve.tensor_tensor(out=mE[pr][:], in0=m_in[sl][:], in1=e_in[sl][:], op=mult).then_inc(dve_p8, 1)
                    dve.tensor_tensor(out=mLV[pr][:], in0=mL[pr][:], in1=mV[pr][:], op=mult).then_inc(dve_p8, 1)
                    dve.tensor_tensor(out=mRV[pr][:], in0=mR[pr][:], in1=mV[pr][:], op=mult).then_inc(dve_p8, 1)
                    dve.tensor_tensor(out=mLR[pr][:], in0=mL[pr][:], in1=mR[pr][:], op=mult).then_inc(dve_p8, 1)

            @block.tensor
            def _(pe):
                pe.wait_ge(const_sem, 32)
                # stat j ready after dve_p8 >= 8u+1+prod_idx[j] (N needs only DMA)
                need = {"N": None, "S1": 2, "SV": 3, "SLP": 4,
                        "SLPV": 6, "SRV": 7, "SLPR": 8}
                ORD = ("N", "S1", "SV", "SLP", "SLPV", "SRV", "SLPR")
                for u in range(NUNIT):
                    sl = u % 3
                    pr = u % 2
                    srcs = {"N": m_in[sl], "S1": mR[pr], "SV": mV[pr],
                            "SLP": mL[pr], "SRV": mRV[pr], "SLPR": mLR[pr],
                            "SLPV": mLV[pr]}
                    for stat in ORD:
                        j = PE_STATS.index(stat)
                        if need[stat] is None:
                            if u == 0:
                                pe.wait_ge(msem0, 16)
                            else:
                                pe.wait_ge(dring[sl], thresh[u])
                        else:
                            pe.wait_ge(dve_p8, 8 * u + need[stat])
                        for k in range(NCH):
                            csl = slice(k * 512, (k + 1) * 512)
                            mm = pe.matmul(
                                st_ps[:],
                                lhsT=oneh_sb[:, j * NPE:(j + 1) * NPE],
                                rhs=srcs[stat][:, csl],
                                start=(u == 0 and stat == ORD[0] and k == 0),
                                stop=(u == NUNIT - 1 and stat == ORD[-1] and k == NCH - 1))
                    mm.then_inc(pe_stat, 1)

            @block.scalar
            def _(act):
                for u in range(NUNIT):
                    pr = u % 2
                    act.wait_ge(dve_p8, 8 * u + 2)
                    act.activation(sq[:], mR[pr][:], Square,
                                   accum_out=cols[:, u:u + 1]).then_inc(act_done, 1)
                    act.wait_ge(dve_p8, 8 * u + 3)
                    act.activation(sq[:], mV[pr][:], Square,
                                   accum_out=cols[:, NUNIT + u:NUNIT + u + 1]) \
                        .then_inc(act_done, 1)
                    act.wait_ge(dve_p8, 8 * u + 5)
                    act.activation(sq[:], mE[pr][:], Copy,
                                   accum_out=cols[:, 2 * NUNIT + u:2 * NUNIT + u + 1]) \
                        .then_inc(act_se, 1)
                act.wait_ge(pe_stat, NUNIT)
                act.activation(stats_sb[:], st_ps[:], Copy).then_inc(act_fin, 1)

    return nc


def _get_program():
    if "nc" not in _cache:
        _cache["nc"] = _build_program()
    return _cache["nc"]


def _tile(x, dtype):
    """(T, BL) shard -> time-reversed, batch-major unit layout (NUNIT*P, W)."""
    # reverse time, transpose to (BL, T)
    xt = x[::-1, :].T                              # (BL, T)
    xt = xt.reshape(NBLK, P, NWIN, W)              # (j, p, w, t)
    xt = np.ascontiguousarray(xt.transpose(0, 2, 1, 3), dtype=np.float32)
    return xt.reshape(NUNIT * P, W).astype(dtype)


def _shard_inputs(inputs):
    import ml_dtypes

    bf16 = ml_dtypes.bfloat16
    fp8 = ml_dtypes.float8_e4m3
    r_t = fp8 if R_DT == "fp8" else bf16
    e_t = fp8 if E_DT == "fp8" else bf16

    r = np.asarray(inputs["rewards"], dtype=np.float32)
    v = np.asarray(inputs["value_estimates"], dtype=np.float32)
    lp = np.asarray(inputs["log_probs"], dtype=np.float32)
    e = np.asarray(inputs["entropies"], dtype=np.float32)
    m = inputs["to_include"].astype(np.float32)
    in_maps = []
    for c in range(NCORES):
        sl = slice(c * BL, (c + 1) * BL)
        in_maps.append({
            "rewards": _tile(r[:, sl], r_t),
            "value_estimates": _tile(v[:, sl], bf16),
            "log_probs": _tile(lp[:, sl], bf16),
            "entropies": _tile(e[:, sl], e_t),
            "to_include": _tile(m[:, sl], bf16),
        })
    return in_maps


def _execute(in_maps, trace=False):
    from concourse.bass_utils import run_bass_kernel_spmd

    nc = _get_program()
    return run_bass_kernel_spmd(nc, in_maps, list(range(NCORES)), trace=trace)


def _stats_from_results(results):
    tot = {name: 0.0 for name in PE_STATS + ("S2", "SV2", "SE")}
    for cm in results:
        pes = cm["pe_stats"].astype(np.float64)
        for j, name in enumerate(PE_STATS):
            tot[name] += pes[j].sum()
        ac = cm["acc_cols"].astype(np.float64)
        tot["S2"] += ac[:, 0:NUNIT].sum()
        tot["SV2"] += ac[:, NUNIT:2 * NUNIT].sum()
        tot["SE"] += ac[:, 2 * NUNIT:3 * NUNIT].sum()
    return tot


def _finalize(tot):
    N = tot["N"]; S1 = tot["S1"]; S2 = tot["S2"]
    SV = tot["SV"]; SRV = tot["SRV"]; SV2 = tot["SV2"]
    SLP = tot["SLP"]; SLPR = tot["SLPR"]; SLPV = tot["SLPV"]; SE = tot["SE"]
    mean = S1 / N
    q = S2 - 2.0 * mean * S1 + mean * mean * N   # sum(m*(R-mean)^2)
    var = q / (N - 1.0)
    s = np.sqrt(var) + EPS
    critic = q / (s * s) - 2.0 * (SRV - mean * SV) / s + SV2
    actor = -(SLPR - mean * SLP) / s + SLPV - ALPHA * SE
    return (np.float32(critic), np.float32(actor))


def kernel(**inputs):
    in_maps = _shard_inputs(inputs)
    res = _execute(in_maps, trace=False)
    tot = _stats_from_results(res.results)
    return _finalize(tot)


# revision 27
# speedup vs baseline: 1.0102x; 1.0102x over previous
"""Trainium2 Bass kernel for nn_ActorCritic loss_fn.

Strategy (batch-major, DVE-scan)
--------------------------------
Both losses are polynomials in 10 masked global sums over the discounted
returns R, values V, log-probs L, entropies E and mask m:

    N    = sum(m)        S1   = sum(m*R)      S2  = sum(m*R^2)
    SV   = sum(m*V)      SRV  = sum(m*R*V)    SV2 = sum(m*V^2)
    SLP  = sum(m*L)      SLPR = sum(m*L*R)    SLPV= sum(m*L*V)
    SE   = sum(m*E)

Layout: batch on SBUF partitions, time along the free dim, TIME-REVERSED
on the host (all sums are order-free, so nothing is un-reversed).  Each
core gets 512 batch columns = 4 partition-blocks of 128; each block's
8192 time steps split into 4 windows of 2048 -> 16 units of (128, 2048)
per core, streamed through a 3-deep input ring.

Engines per unit:
  DVE : the discounted-return scan as ONE native tensor_tensor_scan
        (state = gamma*state + r, fp32 internal state, gamma held as an
        f32 tile so the recurrence coefficient is exact), chained across
        windows via initial=prev_out[:, -1:]; then SEVEN bf16 products
        (mR, mV, mL, mE, mRV, mLR, mLV) in the DVE 2x_1p perf mode
        (2-byte packed operands, ~0.52 ns/col).
  PE  : 7 stat reductions (N,S1,SV,SLP,SRV,SLPR,SLPV) as onehot-column
        matmuls accumulating into one PSUM bank across all 16 units.
  ACT : Square+accum_out for S2/SV2, Copy+accum_out for SE (one column
        per unit; host sums columns).
  GPS : deliberately IDLE - any GpSimd op grabs the SBUF port pair that
        the DVE 2x perf mode needs (exclusive per-instruction lock) and
        stalls the products far more than it contributes (measured).

Rewards travel as fp8e4 (they only feed the scan, whose state is fp32);
everything else bf16 so the DVE products keep 2x mode.  Per-product
semaphores let PE/ACT trail the DVE by one product rather than one unit
(tight pipeline tail), and unit 0's five tensors get dedicated
semaphores so the first scan starts as soon as rewards land.

Raw Bass with manual semaphores (this walrus build allows one sync wait
per instruction -> standalone wait_ge).  Final scalar math on host in
float64 from the 10 sums.
"""

import numpy as np
from contextlib import ExitStack

GAMMA = 0.99
ALPHA = 0.01
EPS = 1e-8

T = 8192
B = 4096
NCORES = 8
BL = B // NCORES          # 512 batch columns per core
P = 128                   # partition dim (batch block)
NBLK = BL // P            # 4 batch blocks
W = 2048                  # time window (free dim per unit)
NWIN = T // W             # 4 windows per block
NUNIT = NBLK * NWIN       # 16 units, u = j*NWIN + w
NCH = W // 512            # 4 matmul chunks per unit (moving max 512)

# dtypes for rewards / entropies ("bf16" or "fp8")
R_DT = "fp8"
E_DT = "bf16"

PE_STATS = ("N", "S1", "SV", "SLP", "SRV", "SLPR", "SLPV")
NPE = len(PE_STATS)

_cache = {}


def _build_program():
    import concourse.bass as bass
    import concourse.mybir as mybir
    import ml_dtypes

    dt = mybir.dt
    f32 = dt.float32
    bf16 = dt.bfloat16
    fp8 = dt.float8e4
    mult = mybir.AluOpType.mult
    add = mybir.AluOpType.add
    Square = mybir.ActivationFunctionType.Square
    Copy = mybir.ActivationFunctionType.Copy

    r_dt = fp8 if R_DT == "fp8" else bf16
    e_dt = fp8 if E_DT == "fp8" else bf16

    nc = bass.Bass()
    r_d = nc.dram_tensor("rewards", [NUNIT * P, W], r_dt, kind="ExternalInput")
    v_d = nc.dram_tensor("value_estimates", [NUNIT * P, W], bf16, kind="ExternalInput")
    l_d = nc.dram_tensor("log_probs", [NUNIT * P, W], bf16, kind="ExternalInput")
    e_d = nc.dram_tensor("entropies", [NUNIT * P, W], e_dt, kind="ExternalInput")
    m_d = nc.dram_tensor("to_include", [NUNIT * P, W], bf16, kind="ExternalInput")
    pes_d = nc.dram_tensor("pe_stats", [NPE, BL], f32, kind="ExternalOutput")
    cols_d = nc.dram_tensor("acc_cols", [P, 3 * NUNIT], f32, kind="ExternalOutput")

    # onehot matrix for stat matmuls: oneh[:, j*NPE + j] = 1
    oneh_np = np.zeros((P, NPE * NPE), dtype=np.float32)
    for j in range(NPE):
        oneh_np[:, j * NPE + j] = 1.0
    oneh_d = nc.inline_tensor(oneh_np.astype(ml_dtypes.bfloat16), "onehmat")
    # gamma tile for the scan (f32 so the recurrence coefficient is exact)
    gam_d = nc.inline_tensor(np.full((P, 2 * W), GAMMA, dtype=np.float32), "gammat")

    with ExitStack() as ctx:
        def sb(name, shape, dtype):
            return ctx.enter_context(nc.sbuf_tensor(name, list(shape), dtype))

        oneh_sb = sb("oneh_sb", (P, NPE * NPE), bf16)
        gam_sb = sb("gam_sb", (P, 2 * W), f32)
        # inputs: 4-slot rings as single tensors (column-sliced) so paired
        # units always see contiguous 2W spans
        r_in = sb("r_in", (P, 4 * W), r_dt)
        v_in = sb("v_in", (P, 4 * W), bf16)
        l_in = sb("l_in", (P, 4 * W), bf16)
        e_in = sb("e_in", (P, 4 * W), e_dt)
        m_in = sb("m_in", (P, 4 * W), bf16)
        R_t = sb("R_t", (P, 2 * W), bf16)
        mR = sb("mR", (P, 2 * W), bf16)
        mV = sb("mV", (P, 2 * W), bf16)
        mL = sb("mL", (P, 2 * W), bf16)
        mRV = sb("mRV", (P, 2 * W), bf16)
        mLR = sb("mLR", (P, 2 * W), bf16)
        mLV = sb("mLV", (P, 2 * W), bf16)
        mE = sb("mE", (P, 2 * W), bf16)
        sq = sb("sq", (P, W), bf16)
        cols = sb("cols", (P, 3 * NUNIT), f32)
        stats_sb = sb("stats_sb", (NPE, BL), f32)
        st_ps = ctx.enter_context(nc.psum_tensor("st_ps", [NPE, BL], f32))

        def isl(u, n=1):
            return slice((u % 4) * W, (u % 4) * W + n * W)

        def psl(u, n=1):
            return slice((u % 2) * W, (u % 2) * W + n * W)

        # unit groups: middle units fused in pairs (even, odd)
        GROUPS = [(0,), (1,)] + [(u, u + 1) for u in range(2, 14, 2)] + [(14,), (15,)]
        PRODS = ("mR", "mV", "mL", "mE", "mLV", "mRV", "mLR")
        POS = {}
        CEND = {}
        opc = 0
        for g in GROUPS:
            opc += 1                      # scan
            for u in g:
                POS[(u, "scan")] = opc
            for nm in PRODS:
                opc += 1
                for u in g:
                    POS[(u, nm)] = opc
            for u in g:
                CEND[u] = opc

        with nc.Block() as block, \
                nc.semaphore("const_sem") as const_sem, \
                nc.semaphore("rsem0") as rsem0, \
                nc.semaphore("rsem0b") as rsem0b, \
                nc.semaphore("msem0") as msem0, \
                nc.semaphore("vsem0") as vsem0, \
                nc.semaphore("lsem0") as lsem0, \
                nc.semaphore("esem0") as esem0, \
                nc.semaphore("dr0") as dr0, \
                nc.semaphore("dr1") as dr1, \
                nc.semaphore("dr2") as dr2, \
                nc.semaphore("dr3") as dr3, \
                nc.semaphore("dve_p8") as dve_p8, \
                nc.semaphore("pe_stat") as pe_stat, \
                nc.semaphore("act_done") as act_done, \
                nc.semaphore("act_se") as act_se, \
                nc.semaphore("act_fin") as act_fin, \
                nc.semaphore("dma_out") as dma_out:
            dring = (dr0, dr1, dr2, dr3)
            thresh = {}
            cnt = [0, 0, 0, 0]
            for u in range(NUNIT):
                cnt[u % 4] += 0 if u == 0 else 80
                thresh[u] = cnt[u % 4]

            @block.sync
            def _(sync):
                sync.dma_start(out=gam_sb[:], in_=gam_d[:]).then_inc(const_sem, 16)
                sync.dma_start(out=oneh_sb[:], in_=oneh_d[:]).then_inc(const_sem, 16)
                sync.dma_start(out=r_in[0:64, isl(0)], in_=r_d[0:64, :]).then_inc(rsem0, 16)
                sync.dma_start(out=r_in[64:128, isl(0)], in_=r_d[64:128, :]).then_inc(rsem0b, 16)
                for dst, src, sem in ((m_in, m_d, msem0), (v_in, v_d, vsem0),
                                      (l_in, l_d, lsem0), (e_in, e_d, esem0)):
                    sync.dma_start(out=dst[:, isl(0)], in_=src[0:P, :]).then_inc(sem, 16)
                for u in range(1, NUNIT):
                    if u >= 4:
                        sync.wait_ge(dve_p8, CEND[u - 4])
                        sync.wait_ge(pe_stat, u - 3)
                    rows = slice(u * P, (u + 1) * P)
                    for dst, src in ((r_in, r_d), (v_in, v_d), (l_in, l_d),
                                     (e_in, e_d), (m_in, m_d)):
                        sync.dma_start(out=dst[:, isl(u)], in_=src[rows, :]) \
                            .then_inc(dring[u % 4], 16)
                sync.wait_ge(act_fin, 1)
                sync.dma_start(out=pes_d[:], in_=stats_sb[:]).then_inc(dma_out, 16)
                sync.wait_ge(act_done, 2 * NUNIT)
                sync.wait_ge(act_se, NUNIT)
                sync.dma_start(out=cols_d[:], in_=cols[:]).then_inc(dma_out, 16)
                sync.wait_ge(dma_out, 32)

            @block.vector
            def _(dve):
                dve.wait_ge(const_sem, 32)
                for g in GROUPS:
                    u0 = g[0]
                    n = len(g)
                    # inputs of every unit in the group
                    if u0 == 0:
                        dve.wait_ge(rsem0, 16)
                        dve.wait_ge(rsem0b, 16)
                    else:
                        for u in g:
                            dve.wait_ge(dring[u % 4], thresh[u])
                    init = 0.0 if u0 % NWIN == 0 else R_t[:, psl(u0 - 1).stop - 1:psl(u0 - 1).stop]
                    dve.tensor_tensor_scan(out=R_t[:, psl(u0, n)],
                                           data0=gam_sb[:, 0:n * W],
                                           data1=r_in[:, isl(u0, n)], initial=init,
                                           op0=mult, op1=add).then_inc(dve_p8, 1)
                    # product-slot WAR: readers of the slots being overwritten
                    if u0 >= 2:
                        dve.wait_ge(pe_stat, u0 + n - 2)
                        dve.wait_ge(act_done, 2 * (u0 + n - 2))
                        dve.wait_ge(act_se, u0 + n - 2)
                    if u0 == 0:
                        dve.wait_ge(msem0, 16)
                    dve.tensor_tensor(out=mR[:, psl(u0, n)], in0=m_in[:, isl(u0, n)],
                                      in1=R_t[:, psl(u0, n)], op=mult).then_inc(dve_p8, 1)
                    if u0 == 0:
                        dve.wait_ge(vsem0, 16)
                    dve.tensor_tensor(out=mV[:, psl(u0, n)], in0=m_in[:, isl(u0, n)],
                                      in1=v_in[:, isl(u0, n)], op=mult).then_inc(dve_p8, 1)
                    if u0 == 0:
                        dve.wait_ge(lsem0, 16)
                    dve.tensor_tensor(out=mL[:, psl(u0, n)], in0=m_in[:, isl(u0, n)],
                                      in1=l_in[:, isl(u0, n)], op=mult).then_inc(dve_p8, 1)
                    if u0 == 0:
                        dve.wait_ge(esem0, 16)
                    dve.tensor_tensor(out=mE[:, psl(u0, n)], in0=m_in[:, isl(u0, n)],
                                      in1=e_in[:, isl(u0, n)], op=mult).then_inc(dve_p8, 1)
                    dve.tensor_tensor(out=mLV[:, psl(u0, n)], in0=mL[:, psl(u0, n)],
                                      in1=mV[:, psl(u0, n)], op=mult).then_inc(dve_p8, 1)
                    dve.tensor_tensor(out=mRV[:, psl(u0, n)], in0=mR[:, psl(u0, n)],
                                      in1=mV[:, psl(u0, n)], op=mult).then_inc(dve_p8, 1)
                    dve.tensor_tensor(out=mLR[:, psl(u0, n)], in0=mL[:, psl(u0, n)],
                                      in1=mR[:, psl(u0, n)], op=mult).then_inc(dve_p8, 1)

            @block.tensor
            def _(pe):
                pe.wait_ge(const_sem, 32)
                need = {"N": None, "S1": "mR", "SV": "mV", "SLP": "mL",
                        "SLPV": "mLV", "SRV": "mRV", "SLPR": "mLR"}
                ORD = ("N", "S1", "SV", "SLP", "SLPV", "SRV", "SLPR")
                for u in range(NUNIT):
                    srcs = {"N": (m_in, isl(u)), "S1": (mR, psl(u)),
                            "SV": (mV, psl(u)), "SLP": (mL, psl(u)),
                            "SRV": (mRV, psl(u)), "SLPR": (mLR, psl(u)),
                            "SLPV": (mLV, psl(u))}
                    for stat in ORD:
                        j = PE_STATS.index(stat)
                        if need[stat] is None:
                            if u == 0:
                                pe.wait_ge(msem0, 16)
                            else:
                                pe.wait_ge(dring[u % 4], thresh[u])
                        else:
                            pe.wait_ge(dve_p8, POS[(u, need[stat])])
                        ten, base = srcs[stat]
                        for k in range(NCH):
                            csl = slice(base.start + k * 512, base.start + (k + 1) * 512)
                            mm = pe.matmul(
                                st_ps[:],
                                lhsT=oneh_sb[:, j * NPE:(j + 1) * NPE],
                                rhs=ten[:, csl],
                                start=(u == 0 and stat == ORD[0] and k == 0),
                                stop=(u == NUNIT - 1 and stat == ORD[-1] and k == NCH - 1))
                    mm.then_inc(pe_stat, 1)

            @block.scalar
            def _(act):
                for u in range(NUNIT):
                    b = psl(u).start
                    act.wait_ge(dve_p8, POS[(u, "mR")])
                    act.activation(sq[:], mR[:, b:b + W], Square,
                                   accum_out=cols[:, u:u + 1]).then_inc(act_done, 1)
                    act.wait_ge(dve_p8, POS[(u, "mV")])
                    act.activation(sq[:], mV[:, b:b + W], Square,
                                   accum_out=cols[:, NUNIT + u:NUNIT + u + 1]) \
                        .then_inc(act_done, 1)
                    act.wait_ge(dve_p8, POS[(u, "mE")])
                    act.activation(sq[:], mE[:, b:b + W], Copy,
                                   accum_out=cols[:, 2 * NUNIT + u:2 * NUNIT + u + 1]) \
                        .then_inc(act_se, 1)
                act.wait_ge(pe_stat, NUNIT)
                act.activation(stats_sb[:], st_ps[:], Copy).then_inc(act_fin, 1)

    return nc


def _get_program():
    if "nc" not in _cache:
        _cache["nc"] = _build_program()
    return _cache["nc"]


def _tile(x, dtype):
    """(T, BL) shard -> time-reversed, batch-major unit layout (NUNIT*P, W)."""
    # reverse time, transpose to (BL, T)
    xt = x[::-1, :].T                              # (BL, T)
    xt = xt.reshape(NBLK, P, NWIN, W)              # (j, p, w, t)
    xt = np.ascontiguousarray(xt.transpose(0, 2, 1, 3), dtype=np.float32)
    return xt.reshape(NUNIT * P, W).astype(dtype)


def _shard_inputs(inputs):
    import ml_dtypes

    bf16 = ml_dtypes.bfloat16
    fp8 = ml_dtypes.float8_e4m3
    r_t = fp8 if R_DT == "fp8" else bf16
    e_t = fp8 if E_DT == "fp8" else bf16

    r = np.asarray(inputs["rewards"], dtype=np.float32)
    v = np.asarray(inputs["value_estimates"], dtype=np.float32)
    lp = np.asarray(inputs["log_probs"], dtype=np.float32)
    e = np.asarray(inputs["entropies"], dtype=np.float32)
    m = inputs["to_include"].astype(np.float32)
    in_maps = []
    for c in range(NCORES):
        sl = slice(c * BL, (c + 1) * BL)
        in_maps.append({
            "rewards": _tile(r[:, sl], r_t),
            "value_estimates": _tile(v[:, sl], bf16),
            "log_probs": _tile(lp[:, sl], bf16),
            "entropies": _tile(e[:, sl], e_t),
            "to_include": _tile(m[:, sl], bf16),
        })
    return in_maps


def _execute(in_maps, trace=False):
    from concourse.bass_utils import run_bass_kernel_spmd

    nc = _get_program()
    return run_bass_kernel_spmd(nc, in_maps, list(range(NCORES)), trace=trace)


def _stats_from_results(results):
    tot = {name: 0.0 for name in PE_STATS + ("S2", "SV2", "SE")}
    for cm in results:
        pes = cm["pe_stats"].astype(np.float64)
        for j, name in enumerate(PE_STATS):
            tot[name] += pes[j].sum()
        ac = cm["acc_cols"].astype(np.float64)
        tot["S2"] += ac[:, 0:NUNIT].sum()
        tot["SV2"] += ac[:, NUNIT:2 * NUNIT].sum()
        tot["SE"] += ac[:, 2 * NUNIT:3 * NUNIT].sum()
    return tot


def _finalize(tot):
    N = tot["N"]; S1 = tot["S1"]; S2 = tot["S2"]
    SV = tot["SV"]; SRV = tot["SRV"]; SV2 = tot["SV2"]
    SLP = tot["SLP"]; SLPR = tot["SLPR"]; SLPV = tot["SLPV"]; SE = tot["SE"]
    mean = S1 / N
    q = S2 - 2.0 * mean * S1 + mean * mean * N   # sum(m*(R-mean)^2)
    var = q / (N - 1.0)
    s = np.sqrt(var) + EPS
    critic = q / (s * s) - 2.0 * (SRV - mean * SV) / s + SV2
    actor = -(SLPR - mean * SLP) / s + SLPV - ALPHA * SE
    return (np.float32(critic), np.float32(actor))


def kernel(**inputs):
    in_maps = _shard_inputs(inputs)
    res = _execute(in_maps, trace=False)
    tot = _stats_from_results(res.results)
    return _finalize(tot)


# revision 28
# speedup vs baseline: 1.0223x; 1.0120x over previous
"""Trainium2 Bass kernel for nn_ActorCritic loss_fn.

Strategy (batch-major, DVE-scan)
--------------------------------
Both losses are polynomials in 10 masked global sums over the discounted
returns R, values V, log-probs L, entropies E and mask m:

    N    = sum(m)        S1   = sum(m*R)      S2  = sum(m*R^2)
    SV   = sum(m*V)      SRV  = sum(m*R*V)    SV2 = sum(m*V^2)
    SLP  = sum(m*L)      SLPR = sum(m*L*R)    SLPV= sum(m*L*V)
    SE   = sum(m*E)

Layout: batch on SBUF partitions, time along the free dim, TIME-REVERSED
on the host (all sums are order-free, so nothing is un-reversed).  Each
core gets 512 batch columns = 4 partition-blocks of 128; each block's
8192 time steps split into 4 windows of 2048 -> 16 units of (128, 2048)
per core, streamed through a 3-deep input ring.

Engines per unit:
  DVE : the discounted-return scan as ONE native tensor_tensor_scan
        (state = gamma*state + r, fp32 internal state, gamma held as an
        f32 tile so the recurrence coefficient is exact), chained across
        windows via initial=prev_out[:, -1:]; then SEVEN bf16 products
        (mR, mV, mL, mE, mRV, mLR, mLV) in the DVE 2x_1p perf mode
        (2-byte packed operands, ~0.52 ns/col).
  PE  : 7 stat reductions (N,S1,SV,SLP,SRV,SLPR,SLPV) as onehot-column
        matmuls accumulating into one PSUM bank across all 16 units.
  ACT : Square+accum_out for S2/SV2, Copy+accum_out for SE (one column
        per unit; host sums columns).
  GPS : deliberately IDLE - any GpSimd op grabs the SBUF port pair that
        the DVE 2x perf mode needs (exclusive per-instruction lock) and
        stalls the products far more than it contributes (measured).

Rewards travel as fp8e4 (they only feed the scan, whose state is fp32);
everything else bf16 so the DVE products keep 2x mode.  Per-product
semaphores let PE/ACT trail the DVE by one product rather than one unit
(tight pipeline tail), and unit 0's five tensors get dedicated
semaphores so the first scan starts as soon as rewards land.

Raw Bass with manual semaphores (this walrus build allows one sync wait
per instruction -> standalone wait_ge).  Final scalar math on host in
float64 from the 10 sums.
"""

import numpy as np
from contextlib import ExitStack

GAMMA = 0.99
ALPHA = 0.01
EPS = 1e-8

T = 8192
B = 4096
NCORES = 8
BL = B // NCORES          # 512 batch columns per core
P = 128                   # partition dim (batch block)
NBLK = BL // P            # 4 batch blocks
W = 2048                  # time window (free dim per unit)
NWIN = T // W             # 4 windows per block
NUNIT = NBLK * NWIN       # 16 units, u = j*NWIN + w
NCH = W // 512            # 4 matmul chunks per unit (moving max 512)

# dtypes for rewards / entropies ("bf16" or "fp8")
R_DT = "fp8"
E_DT = "bf16"

PE_STATS = ("N", "S1", "SV", "SLP", "SRV", "SLPR", "SLPV")
NPE = len(PE_STATS)

_cache = {}


def _build_program():
    import concourse.bass as bass
    import concourse.mybir as mybir
    import ml_dtypes

    dt = mybir.dt
    f32 = dt.float32
    bf16 = dt.bfloat16
    fp8 = dt.float8e4
    mult = mybir.AluOpType.mult
    add = mybir.AluOpType.add
    Square = mybir.ActivationFunctionType.Square
    Copy = mybir.ActivationFunctionType.Copy

    r_dt = fp8 if R_DT == "fp8" else bf16
    e_dt = fp8 if E_DT == "fp8" else bf16

    nc = bass.Bass()
    r_d = nc.dram_tensor("rewards", [NUNIT * P, W], r_dt, kind="ExternalInput")
    v_d = nc.dram_tensor("value_estimates", [NUNIT * P, W], bf16, kind="ExternalInput")
    l_d = nc.dram_tensor("log_probs", [NUNIT * P, W], bf16, kind="ExternalInput")
    e_d = nc.dram_tensor("entropies", [NUNIT * P, W], e_dt, kind="ExternalInput")
    m_d = nc.dram_tensor("to_include", [NUNIT * P, W], bf16, kind="ExternalInput")
    pes_d = nc.dram_tensor("pe_stats", [NPE, BL], f32, kind="ExternalOutput")
    cols_d = nc.dram_tensor("acc_cols", [P, 3 * NUNIT], f32, kind="ExternalOutput")

    # onehot matrix for stat matmuls: oneh[:, j*NPE + j] = 1
    oneh_np = np.zeros((P, NPE * NPE), dtype=np.float32)
    for j in range(NPE):
        oneh_np[:, j * NPE + j] = 1.0
    oneh_d = nc.inline_tensor(oneh_np.astype(ml_dtypes.bfloat16), "onehmat")
    # gamma tile for the scan (f32 so the recurrence coefficient is exact)

    with ExitStack() as ctx:
        def sb(name, shape, dtype):
            return ctx.enter_context(nc.sbuf_tensor(name, list(shape), dtype))

        oneh_sb = sb("oneh_sb", (P, NPE * NPE), bf16)
        gam_sb = sb("gam_sb", (P, 2 * W), f32)
        # inputs: 4-slot rings as single tensors (column-sliced) so paired
        # units always see contiguous 2W spans
        r_in = sb("r_in", (P, 4 * W), r_dt)
        v_in = sb("v_in", (P, 4 * W), bf16)
        l_in = sb("l_in", (P, 4 * W), bf16)
        e_in = sb("e_in", (P, 4 * W), e_dt)
        m_in = sb("m_in", (P, 4 * W), bf16)
        R_t = sb("R_t", (P, 2 * W), bf16)
        mR = sb("mR", (P, 2 * W), bf16)
        mV = sb("mV", (P, 2 * W), bf16)
        mL = sb("mL", (P, 2 * W), bf16)
        mRV = sb("mRV", (P, 2 * W), bf16)
        mLR = sb("mLR", (P, 2 * W), bf16)
        mLV = sb("mLV", (P, 2 * W), bf16)
        mE = sb("mE", (P, 2 * W), bf16)
        sq = sb("sq", (P, W), bf16)
        cols = sb("cols", (P, 3 * NUNIT), f32)
        stats_sb = sb("stats_sb", (NPE, BL), f32)
        st_ps = ctx.enter_context(nc.psum_tensor("st_ps", [NPE, BL], f32))

        def isl(u, n=1):
            return slice((u % 4) * W, (u % 4) * W + n * W)

        def psl(u, n=1):
            return slice((u % 2) * W, (u % 2) * W + n * W)

        # unit groups: middle units fused in pairs (even, odd)
        GROUPS = [(0,), (1,)] + [(u, u + 1) for u in range(2, 14, 2)] + [(14,), (15,)]
        PRODS = ("mR", "mV", "mL", "mE", "mLV", "mRV", "mLR")
        POS = {}
        CEND = {}
        opc = 0
        for g in GROUPS:
            opc += 1                      # scan
            for u in g:
                POS[(u, "scan")] = opc
            for nm in PRODS:
                opc += 1
                for u in g:
                    POS[(u, nm)] = opc
            for u in g:
                CEND[u] = opc

        with nc.Block() as block, \
                nc.semaphore("const_sem") as const_sem, \
                nc.semaphore("gam_sem") as gam_sem, \
                nc.semaphore("rsem0") as rsem0, \
                nc.semaphore("rsem0b") as rsem0b, \
                nc.semaphore("msem0") as msem0, \
                nc.semaphore("vsem0") as vsem0, \
                nc.semaphore("lsem0") as lsem0, \
                nc.semaphore("esem0") as esem0, \
                nc.semaphore("dr0") as dr0, \
                nc.semaphore("dr1") as dr1, \
                nc.semaphore("dr2") as dr2, \
                nc.semaphore("dr3") as dr3, \
                nc.semaphore("dve_p8") as dve_p8, \
                nc.semaphore("pe_stat") as pe_stat, \
                nc.semaphore("act_done") as act_done, \
                nc.semaphore("act_se") as act_se, \
                nc.semaphore("act_fin") as act_fin, \
                nc.semaphore("dma_out") as dma_out:
            dring = (dr0, dr1, dr2, dr3)
            thresh = {}
            cnt = [0, 0, 0, 0]
            for u in range(NUNIT):
                cnt[u % 4] += 0 if u == 0 else 80
                thresh[u] = cnt[u % 4]

            @block.sync
            def _(sync):
                sync.dma_start(out=oneh_sb[:], in_=oneh_d[:]).then_inc(const_sem, 16)
                sync.dma_start(out=r_in[0:64, isl(0)], in_=r_d[0:64, :]).then_inc(rsem0, 16)
                sync.dma_start(out=r_in[64:128, isl(0)], in_=r_d[64:128, :]).then_inc(rsem0b, 16)
                for dst, src, sem in ((m_in, m_d, msem0), (v_in, v_d, vsem0),
                                      (l_in, l_d, lsem0), (e_in, e_d, esem0)):
                    sync.dma_start(out=dst[:, isl(0)], in_=src[0:P, :]).then_inc(sem, 16)
                for u in range(1, NUNIT):
                    if u >= 4:
                        sync.wait_ge(dve_p8, CEND[u - 4])
                        sync.wait_ge(pe_stat, u - 3)
                    rows = slice(u * P, (u + 1) * P)
                    for dst, src in ((r_in, r_d), (v_in, v_d), (l_in, l_d),
                                     (e_in, e_d), (m_in, m_d)):
                        sync.dma_start(out=dst[:, isl(u)], in_=src[rows, :]) \
                            .then_inc(dring[u % 4], 16)
                sync.wait_ge(act_fin, 1)
                sync.dma_start(out=pes_d[:], in_=stats_sb[:]).then_inc(dma_out, 16)
                sync.wait_ge(act_done, 2 * NUNIT)
                sync.wait_ge(act_se, NUNIT)
                sync.dma_start(out=cols_d[:], in_=cols[:]).then_inc(dma_out, 16)
                sync.wait_ge(dma_out, 32)

            @block.gpsimd
            def _(gps):
                gps.memset(gam_sb[:], GAMMA).then_inc(gam_sem, 1)

            @block.vector
            def _(dve):
                dve.wait_ge(gam_sem, 1)
                for g in GROUPS:
                    u0 = g[0]
                    n = len(g)
                    # inputs of every unit in the group
                    if u0 == 0:
                        dve.wait_ge(rsem0, 16)
                        dve.wait_ge(rsem0b, 16)
                    else:
                        for u in g:
                            dve.wait_ge(dring[u % 4], thresh[u])
                    init = 0.0 if u0 % NWIN == 0 else R_t[:, psl(u0 - 1).stop - 1:psl(u0 - 1).stop]
                    dve.tensor_tensor_scan(out=R_t[:, psl(u0, n)],
                                           data0=gam_sb[:, 0:n * W],
                                           data1=r_in[:, isl(u0, n)], initial=init,
                                           op0=mult, op1=add).then_inc(dve_p8, 1)
                    # product-slot WAR: readers of the slots being overwritten
                    if u0 >= 2:
                        dve.wait_ge(pe_stat, u0 + n - 2)
                        dve.wait_ge(act_done, 2 * (u0 + n - 2))
                        dve.wait_ge(act_se, u0 + n - 2)
                    if u0 == 0:
                        dve.wait_ge(msem0, 16)
                    dve.tensor_tensor(out=mR[:, psl(u0, n)], in0=m_in[:, isl(u0, n)],
                                      in1=R_t[:, psl(u0, n)], op=mult).then_inc(dve_p8, 1)
                    if u0 == 0:
                        dve.wait_ge(vsem0, 16)
                    dve.tensor_tensor(out=mV[:, psl(u0, n)], in0=m_in[:, isl(u0, n)],
                                      in1=v_in[:, isl(u0, n)], op=mult).then_inc(dve_p8, 1)
                    if u0 == 0:
                        dve.wait_ge(lsem0, 16)
                    dve.tensor_tensor(out=mL[:, psl(u0, n)], in0=m_in[:, isl(u0, n)],
                                      in1=l_in[:, isl(u0, n)], op=mult).then_inc(dve_p8, 1)
                    if u0 == 0:
                        dve.wait_ge(esem0, 16)
                    dve.tensor_tensor(out=mE[:, psl(u0, n)], in0=m_in[:, isl(u0, n)],
                                      in1=e_in[:, isl(u0, n)], op=mult).then_inc(dve_p8, 1)
                    dve.tensor_tensor(out=mLV[:, psl(u0, n)], in0=mL[:, psl(u0, n)],
                                      in1=mV[:, psl(u0, n)], op=mult).then_inc(dve_p8, 1)
                    dve.tensor_tensor(out=mRV[:, psl(u0, n)], in0=mR[:, psl(u0, n)],
                                      in1=mV[:, psl(u0, n)], op=mult).then_inc(dve_p8, 1)
                    dve.tensor_tensor(out=mLR[:, psl(u0, n)], in0=mL[:, psl(u0, n)],
                                      in1=mR[:, psl(u0, n)], op=mult).then_inc(dve_p8, 1)

            @block.tensor
            def _(pe):
                pe.wait_ge(const_sem, 16)
                need = {"N": None, "S1": "mR", "SV": "mV", "SLP": "mL",
                        "SLPV": "mLV", "SRV": "mRV", "SLPR": "mLR"}
                ORD = ("N", "S1", "SV", "SLP", "SLPV", "SRV", "SLPR")
                for u in range(NUNIT):
                    srcs = {"N": (m_in, isl(u)), "S1": (mR, psl(u)),
                            "SV": (mV, psl(u)), "SLP": (mL, psl(u)),
                            "SRV": (mRV, psl(u)), "SLPR": (mLR, psl(u)),
                            "SLPV": (mLV, psl(u))}
                    for stat in ORD:
                        j = PE_STATS.index(stat)
                        if need[stat] is None:
                            if u == 0:
                                pe.wait_ge(msem0, 16)
                            else:
                                pe.wait_ge(dring[u % 4], thresh[u])
                        else:
                            pe.wait_ge(dve_p8, POS[(u, need[stat])])
                        ten, base = srcs[stat]
                        for k in range(NCH):
                            csl = slice(base.start + k * 512, base.start + (k + 1) * 512)
                            mm = pe.matmul(
                                st_ps[:],
                                lhsT=oneh_sb[:, j * NPE:(j + 1) * NPE],
                                rhs=ten[:, csl],
                                start=(u == 0 and stat == ORD[0] and k == 0),
                                stop=(u == NUNIT - 1 and stat == ORD[-1] and k == NCH - 1))
                    mm.then_inc(pe_stat, 1)

            @block.scalar
            def _(act):
                for u in range(NUNIT):
                    b = psl(u).start
                    act.wait_ge(dve_p8, POS[(u, "mR")])
                    act.activation(sq[:], mR[:, b:b + W], Square,
                                   accum_out=cols[:, u:u + 1]).then_inc(act_done, 1)
                    act.wait_ge(dve_p8, POS[(u, "mV")])
                    act.activation(sq[:], mV[:, b:b + W], Square,
                                   accum_out=cols[:, NUNIT + u:NUNIT + u + 1]) \
                        .then_inc(act_done, 1)
                    act.wait_ge(dve_p8, POS[(u, "mE")])
                    act.activation(sq[:], mE[:, b:b + W], Copy,
                                   accum_out=cols[:, 2 * NUNIT + u:2 * NUNIT + u + 1]) \
                        .then_inc(act_se, 1)
                act.wait_ge(pe_stat, NUNIT)
                act.activation(stats_sb[:], st_ps[:], Copy).then_inc(act_fin, 1)

    return nc


def _get_program():
    if "nc" not in _cache:
        _cache["nc"] = _build_program()
    return _cache["nc"]


def _tile(x, dtype):
    """(T, BL) shard -> time-reversed, batch-major unit layout (NUNIT*P, W)."""
    # reverse time, transpose to (BL, T)
    xt = x[::-1, :].T                              # (BL, T)
    xt = xt.reshape(NBLK, P, NWIN, W)              # (j, p, w, t)
    xt = np.ascontiguousarray(xt.transpose(0, 2, 1, 3), dtype=np.float32)
    return xt.reshape(NUNIT * P, W).astype(dtype)


def _shard_inputs(inputs):
    import ml_dtypes

    bf16 = ml_dtypes.bfloat16
    fp8 = ml_dtypes.float8_e4m3
    r_t = fp8 if R_DT == "fp8" else bf16
    e_t = fp8 if E_DT == "fp8" else bf16

    r = np.asarray(inputs["rewards"], dtype=np.float32)
    v = np.asarray(inputs["value_estimates"], dtype=np.float32)
    lp = np.asarray(inputs["log_probs"], dtype=np.float32)
    e = np.asarray(inputs["entropies"], dtype=np.float32)
    m = inputs["to_include"].astype(np.float32)
    in_maps = []
    for c in range(NCORES):
        sl = slice(c * BL, (c + 1) * BL)
        in_maps.append({
            "rewards": _tile(r[:, sl], r_t),
            "value_estimates": _tile(v[:, sl], bf16),
            "log_probs": _tile(lp[:, sl], bf16),
            "entropies": _tile(e[:, sl], e_t),
            "to_include": _tile(m[:, sl], bf16),
        })
    return in_maps


def _execute(in_maps, trace=False):
    from concourse.bass_utils import run_bass_kernel_spmd

    nc = _get_program()
    return run_bass_kernel_spmd(nc, in_maps, list(range(NCORES)), trace=trace)


def _stats_from_results(results):
    tot = {name: 0.0 for name in PE_STATS + ("S2", "SV2", "SE")}
    for cm in results:
        pes = cm["pe_stats"].astype(np.float64)
        for j, name in enumerate(PE_STATS):
            tot[name] += pes[j].sum()
        ac = cm["acc_cols"].astype(np.float64)
        tot["S2"] += ac[:, 0:NUNIT].sum()
        tot["SV2"] += ac[:, NUNIT:2 * NUNIT].sum()
        tot["SE"] += ac[:, 2 * NUNIT:3 * NUNIT].sum()
    return tot


def _finalize(tot):
    N = tot["N"]; S1 = tot["S1"]; S2 = tot["S2"]
    SV = tot["SV"]; SRV = tot["SRV"]; SV2 = tot["SV2"]
    SLP = tot["SLP"]; SLPR = tot["SLPR"]; SLPV = tot["SLPV"]; SE = tot["SE"]
    mean = S1 / N
    q = S2 - 2.0 * mean * S1 + mean * mean * N   # sum(m*(R-mean)^2)
    var = q / (N - 1.0)
    s = np.sqrt(var) + EPS
    critic = q / (s * s) - 2.0 * (SRV - mean * SV) / s + SV2
    actor = -(SLPR - mean * SLP) / s + SLPV - ALPHA * SE
    return (np.float32(critic), np.float32(actor))


def kernel(**inputs):
    in_maps = _shard_inputs(inputs)
    res = _execute(in_maps, trace=False)
    tot = _stats_from_results(res.results)
    return _finalize(tot)


# revision 29
# speedup vs baseline: 1.0245x; 1.0021x over previous
"""Trainium2 Bass kernel for nn_ActorCritic loss_fn.

Strategy (batch-major, DVE-scan)
--------------------------------
Both losses are polynomials in 10 masked global sums over the discounted
returns R, values V, log-probs L, entropies E and mask m:

    N    = sum(m)        S1   = sum(m*R)      S2  = sum(m*R^2)
    SV   = sum(m*V)      SRV  = sum(m*R*V)    SV2 = sum(m*V^2)
    SLP  = sum(m*L)      SLPR = sum(m*L*R)    SLPV= sum(m*L*V)
    SE   = sum(m*E)

Layout: batch on SBUF partitions, time along the free dim, TIME-REVERSED
on the host (all sums are order-free, so nothing is un-reversed).  Each
core gets 512 batch columns = 4 partition-blocks of 128; each block's
8192 time steps split into 4 windows of 2048 -> 16 units of (128, 2048)
per core, streamed through a 3-deep input ring.

Engines per unit:
  DVE : the discounted-return scan as ONE native tensor_tensor_scan
        (state = gamma*state + r, fp32 internal state, gamma held as an
        f32 tile so the recurrence coefficient is exact), chained across
        windows via initial=prev_out[:, -1:]; then SEVEN bf16 products
        (mR, mV, mL, mE, mRV, mLR, mLV) in the DVE 2x_1p perf mode
        (2-byte packed operands, ~0.52 ns/col).
  PE  : 7 stat reductions (N,S1,SV,SLP,SRV,SLPR,SLPV) as onehot-column
        matmuls accumulating into one PSUM bank across all 16 units.
  ACT : Square+accum_out for S2/SV2, Copy+accum_out for SE (one column
        per unit; host sums columns).
  GPS : deliberately IDLE - any GpSimd op grabs the SBUF port pair that
        the DVE 2x perf mode needs (exclusive per-instruction lock) and
        stalls the products far more than it contributes (measured).

Rewards travel as fp8e4 (they only feed the scan, whose state is fp32);
everything else bf16 so the DVE products keep 2x mode.  Per-product
semaphores let PE/ACT trail the DVE by one product rather than one unit
(tight pipeline tail), and unit 0's five tensors get dedicated
semaphores so the first scan starts as soon as rewards land.

Raw Bass with manual semaphores (this walrus build allows one sync wait
per instruction -> standalone wait_ge).  Final scalar math on host in
float64 from the 10 sums.
"""

import numpy as np
from contextlib import ExitStack

GAMMA = 0.99
ALPHA = 0.01
EPS = 1e-8

T = 8192
B = 4096
NCORES = 8
BL = B // NCORES          # 512 batch columns per core
P = 128                   # partition dim (batch block)
NBLK = BL // P            # 4 batch blocks
W = 2048                  # time window (free dim per unit)
NWIN = T // W             # 4 windows per block
NUNIT = NBLK * NWIN       # 16 units, u = j*NWIN + w
NCH = W // 512            # 4 matmul chunks per unit (moving max 512)

# dtypes for rewards / entropies ("bf16" or "fp8")
R_DT = "fp8"
E_DT = "bf16"

PE_STATS = ("N", "S1", "SV", "SLP", "SRV", "SLPR", "SLPV")
NPE = len(PE_STATS)

_cache = {}


def _build_program():
    import concourse.bass as bass
    import concourse.mybir as mybir
    import ml_dtypes

    dt = mybir.dt
    f32 = dt.float32
    bf16 = dt.bfloat16
    fp8 = dt.float8e4
    mult = mybir.AluOpType.mult
    add = mybir.AluOpType.add
    Square = mybir.ActivationFunctionType.Square
    Copy = mybir.ActivationFunctionType.Copy

    r_dt = fp8 if R_DT == "fp8" else bf16
    e_dt = fp8 if E_DT == "fp8" else bf16

    nc = bass.Bass()
    r_d = nc.dram_tensor("rewards", [NUNIT * P, W], r_dt, kind="ExternalInput")
    v_d = nc.dram_tensor("value_estimates", [NUNIT * P, W], bf16, kind="ExternalInput")
    l_d = nc.dram_tensor("log_probs", [NUNIT * P, W], bf16, kind="ExternalInput")
    e_d = nc.dram_tensor("entropies", [NUNIT * P, W], e_dt, kind="ExternalInput")
    m_d = nc.dram_tensor("to_include", [NUNIT * P, W], bf16, kind="ExternalInput")
    pes_d = nc.dram_tensor("pe_stats", [NPE, BL], f32, kind="ExternalOutput")
    cols_d = nc.dram_tensor("acc_cols", [P, 3 * NUNIT], f32, kind="ExternalOutput")

    # onehot matrix for stat matmuls: oneh[:, j*NPE + j] = 1
    oneh_np = np.zeros((P, NPE * NPE), dtype=np.float32)
    for j in range(NPE):
        oneh_np[:, j * NPE + j] = 1.0
    oneh_d = nc.inline_tensor(oneh_np.astype(ml_dtypes.bfloat16), "onehmat")
    # gamma tile for the scan (f32 so the recurrence coefficient is exact)

    with ExitStack() as ctx:
        def sb(name, shape, dtype):
            return ctx.enter_context(nc.sbuf_tensor(name, list(shape), dtype))

        oneh_sb = sb("oneh_sb", (P, NPE * NPE), bf16)
        gam_sb = sb("gam_sb", (P, 2 * W), f32)
        # inputs: 4-slot rings as single tensors (column-sliced) so paired
        # units always see contiguous 2W spans
        r_in = sb("r_in", (P, 4 * W), r_dt)
        v_in = sb("v_in", (P, 4 * W), bf16)
        l_in = sb("l_in", (P, 4 * W), bf16)
        e_in = sb("e_in", (P, 4 * W), e_dt)
        m_in = sb("m_in", (P, 4 * W), bf16)
        R_t = sb("R_t", (P, 2 * W), bf16)
        mR = sb("mR", (P, 2 * W), bf16)
        mV = sb("mV", (P, 2 * W), bf16)
        mL = sb("mL", (P, 2 * W), bf16)
        mRV = sb("mRV", (P, 2 * W), bf16)
        mLR = sb("mLR", (P, 2 * W), bf16)
        mLV = sb("mLV", (P, 2 * W), bf16)
        mE = sb("mE", (P, 2 * W), bf16)
        sq = sb("sq", (P, W), bf16)
        cols = sb("cols", (P, 3 * NUNIT), f32)
        stats_sb = sb("stats_sb", (NPE, BL), f32)
        st_ps = ctx.enter_context(nc.psum_tensor("st_ps", [NPE, BL], f32))

        def isl(u, n=1):
            return slice((u % 4) * W, (u % 4) * W + n * W)

        def psl(u, n=1):
            return slice((u % 2) * W, (u % 2) * W + n * W)

        # unit groups: middle units fused in pairs (even, odd)
        GROUPS = [(0,), (1,)] + [(u, u + 1) for u in range(2, 14, 2)] + [(14,), (15,)]
        PRODS = ("mR", "mV", "mL", "mE", "mLV", "mRV", "mLR")
        POS = {}
        CEND = {}
        opc = 0
        for g in GROUPS:
            opc += 1                      # scan
            for u in g:
                POS[(u, "scan")] = opc
            for nm in PRODS:
                opc += 1
                for u in g:
                    POS[(u, nm)] = opc
            for u in g:
                CEND[u] = opc

        with nc.Block() as block, \
                nc.semaphore("const_sem") as const_sem, \
                nc.semaphore("gam_sem") as gam_sem, \
                nc.semaphore("rsem0") as rsem0, \
                nc.semaphore("rsem0b") as rsem0b, \
                nc.semaphore("msem0") as msem0, \
                nc.semaphore("vsem0") as vsem0, \
                nc.semaphore("lsem0") as lsem0, \
                nc.semaphore("esem0") as esem0, \
                nc.semaphore("dr0") as dr0, \
                nc.semaphore("dr1") as dr1, \
                nc.semaphore("dr2") as dr2, \
                nc.semaphore("dr3") as dr3, \
                nc.semaphore("dve_p8") as dve_p8, \
                nc.semaphore("pe_stat") as pe_stat, \
                nc.semaphore("act_done") as act_done, \
                nc.semaphore("act_se") as act_se, \
                nc.semaphore("act_fin") as act_fin, \
                nc.semaphore("dma_out") as dma_out:
            dring = (dr0, dr1, dr2, dr3)
            thresh = {}
            cnt = [0, 0, 0, 0]
            for u in range(NUNIT):
                cnt[u % 4] += 0 if u == 0 else 80
                thresh[u] = cnt[u % 4]

            @block.sync
            def _(sync):
                sync.dma_start(out=oneh_sb[:], in_=oneh_d[:]).then_inc(const_sem, 16)
                sync.dma_start(out=r_in[0:64, isl(0)], in_=r_d[0:64, :]).then_inc(rsem0, 16)
                sync.dma_start(out=r_in[64:128, isl(0)], in_=r_d[64:128, :]).then_inc(rsem0b, 16)
                for dst, src, sem in ((m_in, m_d, msem0), (v_in, v_d, vsem0),
                                      (l_in, l_d, lsem0), (e_in, e_d, esem0)):
                    sync.dma_start(out=dst[:, isl(0)], in_=src[0:P, :]).then_inc(sem, 16)
                for u in range(1, NUNIT):
                    if u >= 4:
                        sync.wait_ge(dve_p8, CEND[u - 4])
                        sync.wait_ge(pe_stat, u - 3)
                    rows = slice(u * P, (u + 1) * P)
                    for dst, src in ((r_in, r_d), (m_in, m_d), (v_in, v_d),
                                     (l_in, l_d), (e_in, e_d)):
                        sync.dma_start(out=dst[:, isl(u)], in_=src[rows, :]) \
                            .then_inc(dring[u % 4], 16)
                sync.wait_ge(act_fin, 1)
                sync.dma_start(out=pes_d[:], in_=stats_sb[:]).then_inc(dma_out, 16)
                sync.wait_ge(act_done, 2 * NUNIT)
                sync.wait_ge(act_se, NUNIT)
                sync.dma_start(out=cols_d[:], in_=cols[:]).then_inc(dma_out, 16)
                sync.wait_ge(dma_out, 32)

            @block.gpsimd
            def _(gps):
                gps.memset(gam_sb[:], GAMMA).then_inc(gam_sem, 1)

            @block.vector
            def _(dve):
                dve.wait_ge(gam_sem, 1)
                for g in GROUPS:
                    u0 = g[0]
                    n = len(g)
                    # inputs of every unit in the group
                    if u0 == 0:
                        dve.wait_ge(rsem0, 16)
                        dve.wait_ge(rsem0b, 16)
                    else:
                        for u in g:
                            dve.wait_ge(dring[u % 4], thresh[u])
                    init = 0.0 if u0 % NWIN == 0 else R_t[:, psl(u0 - 1).stop - 1:psl(u0 - 1).stop]
                    dve.tensor_tensor_scan(out=R_t[:, psl(u0, n)],
                                           data0=gam_sb[:, 0:n * W],
                                           data1=r_in[:, isl(u0, n)], initial=init,
                                           op0=mult, op1=add).then_inc(dve_p8, 1)
                    # product-slot WAR: readers of the slots being overwritten
                    if u0 >= 2:
                        dve.wait_ge(pe_stat, u0 + n - 2)
                        dve.wait_ge(act_done, 2 * (u0 + n - 2))
                        dve.wait_ge(act_se, u0 + n - 2)
                    if u0 == 0:
                        dve.wait_ge(msem0, 16)
                    dve.tensor_tensor(out=mR[:, psl(u0, n)], in0=m_in[:, isl(u0, n)],
                                      in1=R_t[:, psl(u0, n)], op=mult).then_inc(dve_p8, 1)
                    if u0 == 0:
                        dve.wait_ge(vsem0, 16)
                    dve.tensor_tensor(out=mV[:, psl(u0, n)], in0=m_in[:, isl(u0, n)],
                                      in1=v_in[:, isl(u0, n)], op=mult).then_inc(dve_p8, 1)
                    if u0 == 0:
                        dve.wait_ge(lsem0, 16)
                    dve.tensor_tensor(out=mL[:, psl(u0, n)], in0=m_in[:, isl(u0, n)],
                                      in1=l_in[:, isl(u0, n)], op=mult).then_inc(dve_p8, 1)
                    if u0 == 0:
                        dve.wait_ge(esem0, 16)
                    dve.tensor_tensor(out=mE[:, psl(u0, n)], in0=m_in[:, isl(u0, n)],
                                      in1=e_in[:, isl(u0, n)], op=mult).then_inc(dve_p8, 1)
                    dve.tensor_tensor(out=mLV[:, psl(u0, n)], in0=mL[:, psl(u0, n)],
                                      in1=mV[:, psl(u0, n)], op=mult).then_inc(dve_p8, 1)
                    dve.tensor_tensor(out=mRV[:, psl(u0, n)], in0=mR[:, psl(u0, n)],
                                      in1=mV[:, psl(u0, n)], op=mult).then_inc(dve_p8, 1)
                    dve.tensor_tensor(out=mLR[:, psl(u0, n)], in0=mL[:, psl(u0, n)],
                                      in1=mR[:, psl(u0, n)], op=mult).then_inc(dve_p8, 1)

            @block.tensor
            def _(pe):
                pe.wait_ge(const_sem, 16)
                need = {"N": None, "S1": "mR", "SV": "mV", "SLP": "mL",
                        "SLPV": "mLV", "SRV": "mRV", "SLPR": "mLR"}
                ORD = ("N", "S1", "SV", "SLP", "SLPV", "SRV", "SLPR")
                for u in range(NUNIT):
                    srcs = {"N": (m_in, isl(u)), "S1": (mR, psl(u)),
                            "SV": (mV, psl(u)), "SLP": (mL, psl(u)),
                            "SRV": (mRV, psl(u)), "SLPR": (mLR, psl(u)),
                            "SLPV": (mLV, psl(u))}
                    for stat in ORD:
                        j = PE_STATS.index(stat)
                        if need[stat] is None:
                            if u == 0:
                                pe.wait_ge(msem0, 16)
                            else:
                                pe.wait_ge(dring[u % 4], thresh[u])
                        else:
                            pe.wait_ge(dve_p8, POS[(u, need[stat])])
                        ten, base = srcs[stat]
                        for k in range(NCH):
                            csl = slice(base.start + k * 512, base.start + (k + 1) * 512)
                            mm = pe.matmul(
                                st_ps[:],
                                lhsT=oneh_sb[:, j * NPE:(j + 1) * NPE],
                                rhs=ten[:, csl],
                                start=(u == 0 and stat == ORD[0] and k == 0),
                                stop=(u == NUNIT - 1 and stat == ORD[-1] and k == NCH - 1))
                    mm.then_inc(pe_stat, 1)

            @block.scalar
            def _(act):
                for u in range(NUNIT):
                    b = psl(u).start
                    act.wait_ge(dve_p8, POS[(u, "mR")])
                    act.activation(sq[:], mR[:, b:b + W], Square,
                                   accum_out=cols[:, u:u + 1]).then_inc(act_done, 1)
                    act.wait_ge(dve_p8, POS[(u, "mV")])
                    act.activation(sq[:], mV[:, b:b + W], Square,
                                   accum_out=cols[:, NUNIT + u:NUNIT + u + 1]) \
                        .then_inc(act_done, 1)
                    act.wait_ge(dve_p8, POS[(u, "mE")])
                    act.activation(sq[:], mE[:, b:b + W], Copy,
                                   accum_out=cols[:, 2 * NUNIT + u:2 * NUNIT + u + 1]) \
                        .then_inc(act_se, 1)
                act.wait_ge(pe_stat, NUNIT)
                act.activation(stats_sb[:], st_ps[:], Copy).then_inc(act_fin, 1)

    return nc


def _get_program():
    if "nc" not in _cache:
        _cache["nc"] = _build_program()
    return _cache["nc"]


def _tile(x, dtype):
    """(T, BL) shard -> time-reversed, batch-major unit layout (NUNIT*P, W)."""
    # reverse time, transpose to (BL, T)
    xt = x[::-1, :].T                              # (BL, T)
    xt = xt.reshape(NBLK, P, NWIN, W)              # (j, p, w, t)
    xt = np.ascontiguousarray(xt.transpose(0, 2, 1, 3), dtype=np.float32)
    return xt.reshape(NUNIT * P, W).astype(dtype)


def _shard_inputs(inputs):
    import ml_dtypes

    bf16 = ml_dtypes.bfloat16
    fp8 = ml_dtypes.float8_e4m3
    r_t = fp8 if R_DT == "fp8" else bf16
    e_t = fp8 if E_DT == "fp8" else bf16

    r = np.asarray(inputs["rewards"], dtype=np.float32)
    v = np.asarray(inputs["value_estimates"], dtype=np.float32)
    lp = np.asarray(inputs["log_probs"], dtype=np.float32)
    e = np.asarray(inputs["entropies"], dtype=np.float32)
    m = inputs["to_include"].astype(np.float32)
    in_maps = []
    for c in range(NCORES):
        sl = slice(c * BL, (c + 1) * BL)
        in_maps.append({
            "rewards": _tile(r[:, sl], r_t),
            "value_estimates": _tile(v[:, sl], bf16),
            "log_probs": _tile(lp[:, sl], bf16),
            "entropies": _tile(e[:, sl], e_t),
            "to_include": _tile(m[:, sl], bf16),
        })
    return in_maps


def _execute(in_maps, trace=False):
    from concourse.bass_utils import run_bass_kernel_spmd

    nc = _get_program()
    return run_bass_kernel_spmd(nc, in_maps, list(range(NCORES)), trace=trace)


def _stats_from_results(results):
    tot = {name: 0.0 for name in PE_STATS + ("S2", "SV2", "SE")}
    for cm in results:
        pes = cm["pe_stats"].astype(np.float64)
        for j, name in enumerate(PE_STATS):
            tot[name] += pes[j].sum()
        ac = cm["acc_cols"].astype(np.float64)
        tot["S2"] += ac[:, 0:NUNIT].sum()
        tot["SV2"] += ac[:, NUNIT:2 * NUNIT].sum()
        tot["SE"] += ac[:, 2 * NUNIT:3 * NUNIT].sum()
    return tot


def _finalize(tot):
    N = tot["N"]; S1 = tot["S1"]; S2 = tot["S2"]
    SV = tot["SV"]; SRV = tot["SRV"]; SV2 = tot["SV2"]
    SLP = tot["SLP"]; SLPR = tot["SLPR"]; SLPV = tot["SLPV"]; SE = tot["SE"]
    mean = S1 / N
    q = S2 - 2.0 * mean * S1 + mean * mean * N   # sum(m*(R-mean)^2)
    var = q / (N - 1.0)
    s = np.sqrt(var) + EPS
    critic = q / (s * s) - 2.0 * (SRV - mean * SV) / s + SV2
    actor = -(SLPR - mean * SLP) / s + SLPV - ALPHA * SE
    return (np.float32(critic), np.float32(actor))


def kernel(**inputs):
    in_maps = _shard_inputs(inputs)
    res = _execute(in_maps, trace=False)
    tot = _stats_from_results(res.results)
    return _finalize(tot)


# revision 30
# speedup vs baseline: 1.0414x; 1.0165x over previous
"""Trainium2 Bass kernel for nn_ActorCritic loss_fn.

Strategy (batch-major, DVE-scan)
--------------------------------
Both losses are polynomials in 10 masked global sums over the discounted
returns R, values V, log-probs L, entropies E and mask m:

    N    = sum(m)        S1   = sum(m*R)      S2  = sum(m*R^2)
    SV   = sum(m*V)      SRV  = sum(m*R*V)    SV2 = sum(m*V^2)
    SLP  = sum(m*L)      SLPR = sum(m*L*R)    SLPV= sum(m*L*V)
    SE   = sum(m*E)

Layout: batch on SBUF partitions, time along the free dim, TIME-REVERSED
on the host (all sums are order-free, so nothing is un-reversed).  Each
core gets 512 batch columns = 4 partition-blocks of 128; each block's
8192 time steps split into 4 windows of 2048 -> 16 units of (128, 2048)
per core, streamed through a 3-deep input ring.

Engines per unit:
  DVE : the discounted-return scan as ONE native tensor_tensor_scan
        (state = gamma*state + r, fp32 internal state, gamma held as an
        f32 tile so the recurrence coefficient is exact), chained across
        windows via initial=prev_out[:, -1:]; then SEVEN bf16 products
        (mR, mV, mL, mE, mRV, mLR, mLV) in the DVE 2x_1p perf mode
        (2-byte packed operands, ~0.52 ns/col).
  PE  : 7 stat reductions (N,S1,SV,SLP,SRV,SLPR,SLPV) as onehot-column
        matmuls accumulating into one PSUM bank across all 16 units.
  ACT : Square+accum_out for S2/SV2, Copy+accum_out for SE (one column
        per unit; host sums columns).
  GPS : deliberately IDLE - any GpSimd op grabs the SBUF port pair that
        the DVE 2x perf mode needs (exclusive per-instruction lock) and
        stalls the products far more than it contributes (measured).

Rewards travel as fp8e4 (they only feed the scan, whose state is fp32);
everything else bf16 so the DVE products keep 2x mode.  Per-product
semaphores let PE/ACT trail the DVE by one product rather than one unit
(tight pipeline tail), and unit 0's five tensors get dedicated
semaphores so the first scan starts as soon as rewards land.

Raw Bass with manual semaphores (this walrus build allows one sync wait
per instruction -> standalone wait_ge).  Final scalar math on host in
float64 from the 10 sums.
"""

import numpy as np
from contextlib import ExitStack

GAMMA = 0.99
ALPHA = 0.01
EPS = 1e-8

T = 8192
B = 4096
NCORES = 8
BL = B // NCORES          # 512 batch columns per core
P = 128                   # partition dim (batch block)
NBLK = BL // P            # 4 batch blocks
W = 2048                  # time window (free dim per unit)
NWIN = T // W             # 4 windows per block
NUNIT = NBLK * NWIN       # 16 units, u = j*NWIN + w
NCH = W // 512            # 4 matmul chunks per unit (moving max 512)

# dtypes for rewards / entropies ("bf16" or "fp8")
R_DT = "fp8"
E_DT = "bf16"

PE_STATS = ("N", "S1", "SV", "SLP", "SRV", "SLPR", "SLPV")
NPE = len(PE_STATS)

_cache = {}


def _build_program():
    import concourse.bass as bass
    import concourse.mybir as mybir
    import ml_dtypes

    dt = mybir.dt
    f32 = dt.float32
    bf16 = dt.bfloat16
    fp8 = dt.float8e4
    mult = mybir.AluOpType.mult
    add = mybir.AluOpType.add
    Square = mybir.ActivationFunctionType.Square
    Copy = mybir.ActivationFunctionType.Copy

    r_dt = fp8 if R_DT == "fp8" else bf16
    e_dt = fp8 if E_DT == "fp8" else bf16

    nc = bass.Bass()
    r_d = nc.dram_tensor("rewards", [NUNIT * P, W], r_dt, kind="ExternalInput")
    v_d = nc.dram_tensor("value_estimates", [NUNIT * P, W], bf16, kind="ExternalInput")
    l_d = nc.dram_tensor("log_probs", [NUNIT * P, W], bf16, kind="ExternalInput")
    e_d = nc.dram_tensor("entropies", [NUNIT * P, W], e_dt, kind="ExternalInput")
    m_d = nc.dram_tensor("to_include", [NUNIT * P, W], bf16, kind="ExternalInput")
    pes_d = nc.dram_tensor("pe_stats", [NPE, BL], f32, kind="ExternalOutput")
    cols_d = nc.dram_tensor("acc_cols", [P, 3 * NUNIT], f32, kind="ExternalOutput")

    # onehot matrix for stat matmuls: oneh[:, j*NPE + j] = 1
    oneh_np = np.zeros((P, NPE * NPE), dtype=np.float32)
    for j in range(NPE):
        oneh_np[:, j * NPE + j] = 1.0
    oneh_d = nc.inline_tensor(oneh_np.astype(ml_dtypes.bfloat16), "onehmat")
    # gamma tile for the scan (f32 so the recurrence coefficient is exact)

    with ExitStack() as ctx:
        def sb(name, shape, dtype):
            return ctx.enter_context(nc.sbuf_tensor(name, list(shape), dtype))

        oneh_sb = sb("oneh_sb", (P, NPE * NPE), bf16)
        gam_sb = sb("gam_sb", (P, 2 * W), f32)
        # inputs: 4-slot rings as single tensors (column-sliced) so paired
        # units always see contiguous 2W spans
        r_in = sb("r_in", (P, 4 * W), r_dt)
        v_in = sb("v_in", (P, 4 * W), bf16)
        l_in = sb("l_in", (P, 4 * W), bf16)
        e_in = sb("e_in", (P, 4 * W), e_dt)
        m_in = sb("m_in", (P, 4 * W), bf16)
        R_t = sb("R_t", (P, 2 * W), bf16)
        mR = sb("mR", (P, 2 * W), bf16)
        mV = sb("mV", (P, 2 * W), bf16)
        mL = sb("mL", (P, 2 * W), bf16)
        mRV = sb("mRV", (P, 2 * W), bf16)
        mLR = sb("mLR", (P, 2 * W), bf16)
        mLV = sb("mLV", (P, 2 * W), bf16)
        mE = sb("mE", (P, 2 * W), bf16)
        sq = sb("sq", (P, W), bf16)
        cols = sb("cols", (P, 3 * NUNIT), f32)
        stats_sb = sb("stats_sb", (NPE, BL), f32)
        st_ps = ctx.enter_context(nc.psum_tensor("st_ps", [NPE, BL], f32))

        def isl(u, n=1):
            return slice((u % 4) * W, (u % 4) * W + n * W)

        def psl(u, n=1):
            return slice((u % 2) * W, (u % 2) * W + n * W)

        # unit groups: middle units fused in pairs (even, odd)
        GROUPS = [(0,), (1,)] + [(u, u + 1) for u in range(2, 14, 2)] + [(14,), (15,)]
        PRODS = ("mR", "mV", "mL", "mE", "mLV", "mRV", "mLR")
        POS = {}
        CEND = {}
        opc = 0
        for g in GROUPS:
            opc += 1                      # scan
            for u in g:
                POS[(u, "scan")] = opc
            for nm in PRODS:
                opc += 1
                for u in g:
                    POS[(u, nm)] = opc
            for u in g:
                CEND[u] = opc

        with nc.Block() as block, \
                nc.semaphore("const_sem") as const_sem, \
                nc.semaphore("gam_sem") as gam_sem, \
                nc.semaphore("rsem0") as rsem0, \
                nc.semaphore("rsem0b") as rsem0b, \
                nc.semaphore("msem0") as msem0, \
                nc.semaphore("vsem0") as vsem0, \
                nc.semaphore("lsem0") as lsem0, \
                nc.semaphore("esem0") as esem0, \
                nc.semaphore("dr0") as dr0, \
                nc.semaphore("dr1") as dr1, \
                nc.semaphore("dr2") as dr2, \
                nc.semaphore("dr3") as dr3, \
                nc.semaphore("dve_p8") as dve_p8, \
                nc.semaphore("pe_stat") as pe_stat, \
                nc.semaphore("act_done") as act_done, \
                nc.semaphore("act_se") as act_se, \
                nc.semaphore("act_fin") as act_fin, \
                nc.semaphore("dma_out") as dma_out:
            dring = (dr0, dr1, dr2, dr3)
            thresh = {}
            cnt = [0, 0, 0, 0]
            for u in range(NUNIT):
                cnt[u % 4] += 0 if u == 0 else 80
                thresh[u] = cnt[u % 4]

            @block.sync
            def _(sync):
                sync.dma_start(out=oneh_sb[:], in_=oneh_d[:]).then_inc(const_sem, 16)
                sync.dma_start(out=r_in[0:64, isl(0)], in_=r_d[0:64, :]).then_inc(rsem0, 16)
                sync.dma_start(out=r_in[64:128, isl(0)], in_=r_d[64:128, :]).then_inc(rsem0b, 16)
                for dst, src, sem in ((m_in, m_d, msem0), (v_in, v_d, vsem0),
                                      (l_in, l_d, lsem0), (e_in, e_d, esem0)):
                    sync.dma_start(out=dst[:, isl(0)], in_=src[0:P, :]).then_inc(sem, 16)
                for u in range(1, NUNIT):
                    if u >= 4:
                        sync.wait_ge(dve_p8, CEND[u - 4])
                        sync.wait_ge(pe_stat, u - 3)
                    rows = slice(u * P, (u + 1) * P)
                    for dst, src in ((r_in, r_d), (m_in, m_d), (v_in, v_d),
                                     (l_in, l_d), (e_in, e_d)):
                        sync.dma_start(out=dst[:, isl(u)], in_=src[rows, :]) \
                            .then_inc(dring[u % 4], 16)
                sync.wait_ge(act_done, 2 * (NUNIT - 2))
                sync.wait_ge(act_se, NUNIT - 2)
                sync.dma_start(out=cols_d[:, 0:42], in_=cols[:, 0:42]).then_inc(dma_out, 16)
                sync.wait_ge(act_done, 2 * NUNIT)
                sync.wait_ge(act_se, NUNIT)
                sync.dma_start(out=cols_d[:, 42:48], in_=cols[:, 42:48]).then_inc(dma_out, 16)
                sync.wait_ge(act_fin, 1)
                sync.dma_start(out=pes_d[:], in_=stats_sb[:]).then_inc(dma_out, 16)
                sync.wait_ge(dma_out, 48)

            @block.gpsimd
            def _(gps):
                gps.memset(gam_sb[:], GAMMA).then_inc(gam_sem, 1)

            @block.vector
            def _(dve):
                dve.wait_ge(gam_sem, 1)
                for g in GROUPS:
                    u0 = g[0]
                    n = len(g)
                    # inputs of every unit in the group
                    if u0 == 0:
                        dve.wait_ge(rsem0, 16)
                        dve.wait_ge(rsem0b, 16)
                    else:
                        for u in g:
                            dve.wait_ge(dring[u % 4], thresh[u])
                    init = 0.0 if u0 % NWIN == 0 else R_t[:, psl(u0 - 1).stop - 1:psl(u0 - 1).stop]
                    dve.tensor_tensor_scan(out=R_t[:, psl(u0, n)],
                                           data0=gam_sb[:, 0:n * W],
                                           data1=r_in[:, isl(u0, n)], initial=init,
                                           op0=mult, op1=add).then_inc(dve_p8, 1)
                    # product-slot WAR: readers of the slots being overwritten
                    if u0 >= 2:
                        dve.wait_ge(pe_stat, u0 + n - 2)
                        dve.wait_ge(act_done, 2 * (u0 + n - 2))
                        dve.wait_ge(act_se, u0 + n - 2)
                    if u0 == 0:
                        dve.wait_ge(msem0, 16)
                    dve.tensor_tensor(out=mR[:, psl(u0, n)], in0=m_in[:, isl(u0, n)],
                                      in1=R_t[:, psl(u0, n)], op=mult).then_inc(dve_p8, 1)
                    if u0 == 0:
                        dve.wait_ge(vsem0, 16)
                    dve.tensor_tensor(out=mV[:, psl(u0, n)], in0=m_in[:, isl(u0, n)],
                                      in1=v_in[:, isl(u0, n)], op=mult).then_inc(dve_p8, 1)
                    if u0 == 0:
                        dve.wait_ge(lsem0, 16)
                    dve.tensor_tensor(out=mL[:, psl(u0, n)], in0=m_in[:, isl(u0, n)],
                                      in1=l_in[:, isl(u0, n)], op=mult).then_inc(dve_p8, 1)
                    if u0 == 0:
                        dve.wait_ge(esem0, 16)
                    dve.tensor_tensor(out=mE[:, psl(u0, n)], in0=m_in[:, isl(u0, n)],
                                      in1=e_in[:, isl(u0, n)], op=mult).then_inc(dve_p8, 1)
                    dve.tensor_tensor(out=mLV[:, psl(u0, n)], in0=mL[:, psl(u0, n)],
                                      in1=mV[:, psl(u0, n)], op=mult).then_inc(dve_p8, 1)
                    dve.tensor_tensor(out=mRV[:, psl(u0, n)], in0=mR[:, psl(u0, n)],
                                      in1=mV[:, psl(u0, n)], op=mult).then_inc(dve_p8, 1)
                    dve.tensor_tensor(out=mLR[:, psl(u0, n)], in0=mL[:, psl(u0, n)],
                                      in1=mR[:, psl(u0, n)], op=mult).then_inc(dve_p8, 1)

            @block.tensor
            def _(pe):
                pe.wait_ge(const_sem, 16)
                need = {"N": None, "S1": "mR", "SV": "mV", "SLP": "mL",
                        "SLPV": "mLV", "SRV": "mRV", "SLPR": "mLR"}
                ORD = ("N", "S1", "SV", "SLP", "SLPV", "SRV", "SLPR")
                for u in range(NUNIT):
                    srcs = {"N": (m_in, isl(u)), "S1": (mR, psl(u)),
                            "SV": (mV, psl(u)), "SLP": (mL, psl(u)),
                            "SRV": (mRV, psl(u)), "SLPR": (mLR, psl(u)),
                            "SLPV": (mLV, psl(u))}
                    for stat in ORD:
                        j = PE_STATS.index(stat)
                        if need[stat] is None:
                            if u == 0:
                                pe.wait_ge(msem0, 16)
                            else:
                                pe.wait_ge(dring[u % 4], thresh[u])
                        else:
                            pe.wait_ge(dve_p8, POS[(u, need[stat])])
                        ten, base = srcs[stat]
                        for k in range(NCH):
                            csl = slice(base.start + k * 512, base.start + (k + 1) * 512)
                            mm = pe.matmul(
                                st_ps[:],
                                lhsT=oneh_sb[:, j * NPE:(j + 1) * NPE],
                                rhs=ten[:, csl],
                                start=(u == 0 and stat == ORD[0] and k == 0),
                                stop=(u == NUNIT - 1 and stat == ORD[-1] and k == NCH - 1))
                    mm.then_inc(pe_stat, 1)

            @block.scalar
            def _(act):
                for u in range(NUNIT):
                    b = psl(u).start
                    act.wait_ge(dve_p8, POS[(u, "mR")])
                    act.activation(sq[:], mR[:, b:b + W], Square,
                                   accum_out=cols[:, 3 * u:3 * u + 1]).then_inc(act_done, 1)
                    act.wait_ge(dve_p8, POS[(u, "mV")])
                    act.activation(sq[:], mV[:, b:b + W], Square,
                                   accum_out=cols[:, 3 * u + 1:3 * u + 2]) \
                        .then_inc(act_done, 1)
                    act.wait_ge(dve_p8, POS[(u, "mE")])
                    act.activation(sq[:], mE[:, b:b + W], Copy,
                                   accum_out=cols[:, 3 * u + 2:3 * u + 3]) \
                        .then_inc(act_se, 1)
                act.wait_ge(pe_stat, NUNIT)
                act.activation(stats_sb[:], st_ps[:], Copy).then_inc(act_fin, 1)

    return nc


def _get_program():
    if "nc" not in _cache:
        _cache["nc"] = _build_program()
    return _cache["nc"]


def _tile(x, dtype):
    """(T, BL) shard -> time-reversed, batch-major unit layout (NUNIT*P, W)."""
    # reverse time, transpose to (BL, T)
    xt = x[::-1, :].T                              # (BL, T)
    xt = xt.reshape(NBLK, P, NWIN, W)              # (j, p, w, t)
    xt = np.ascontiguousarray(xt.transpose(0, 2, 1, 3), dtype=np.float32)
    return xt.reshape(NUNIT * P, W).astype(dtype)


def _shard_inputs(inputs):
    import ml_dtypes

    bf16 = ml_dtypes.bfloat16
    fp8 = ml_dtypes.float8_e4m3
    r_t = fp8 if R_DT == "fp8" else bf16
    e_t = fp8 if E_DT == "fp8" else bf16

    r = np.asarray(inputs["rewards"], dtype=np.float32)
    v = np.asarray(inputs["value_estimates"], dtype=np.float32)
    lp = np.asarray(inputs["log_probs"], dtype=np.float32)
    e = np.asarray(inputs["entropies"], dtype=np.float32)
    m = inputs["to_include"].astype(np.float32)
    in_maps = []
    for c in range(NCORES):
        sl = slice(c * BL, (c + 1) * BL)
        in_maps.append({
            "rewards": _tile(r[:, sl], r_t),
            "value_estimates": _tile(v[:, sl], bf16),
            "log_probs": _tile(lp[:, sl], bf16),
            "entropies": _tile(e[:, sl], e_t),
            "to_include": _tile(m[:, sl], bf16),
        })
    return in_maps


def _execute(in_maps, trace=False):
    from concourse.bass_utils import run_bass_kernel_spmd

    nc = _get_program()
    return run_bass_kernel_spmd(nc, in_maps, list(range(NCORES)), trace=trace)


def _stats_from_results(results):
    tot = {name: 0.0 for name in PE_STATS + ("S2", "SV2", "SE")}
    for cm in results:
        pes = cm["pe_stats"].astype(np.float64)
        for j, name in enumerate(PE_STATS):
            tot[name] += pes[j].sum()
        ac = cm["acc_cols"].astype(np.float64)
        tot["S2"] += ac[:, 0::3].sum()
        tot["SV2"] += ac[:, 1::3].sum()
        tot["SE"] += ac[:, 2::3].sum()
    return tot


def _finalize(tot):
    N = tot["N"]; S1 = tot["S1"]; S2 = tot["S2"]
    SV = tot["SV"]; SRV = tot["SRV"]; SV2 = tot["SV2"]
    SLP = tot["SLP"]; SLPR = tot["SLPR"]; SLPV = tot["SLPV"]; SE = tot["SE"]
    mean = S1 / N
    q = S2 - 2.0 * mean * S1 + mean * mean * N   # sum(m*(R-mean)^2)
    var = q / (N - 1.0)
    s = np.sqrt(var) + EPS
    critic = q / (s * s) - 2.0 * (SRV - mean * SV) / s + SV2
    actor = -(SLPR - mean * SLP) / s + SLPV - ALPHA * SE
    return (np.float32(critic), np.float32(actor))


def kernel(**inputs):
    in_maps = _shard_inputs(inputs)
    res = _execute(in_maps, trace=False)
    tot = _stats_from_results(res.results)
    return _finalize(tot)
